# revision 23
# baseline (speedup 1.0000x reference)
"""BIDAF attention-flow kernel for Trainium2 (Bass/Tile), 8-core data-parallel.

Reference computation (per batch b):
    S[t,j]  = H[t]·w_h + U[j]·w_u + sum_d H[t,d]*U[j,d]*w_hu[d]
    A       = softmax_j(S);          C2Q = A @ U
    b_att   = softmax_t(max_j S);    Q2C = b_att @ H   (broadcast over t)
    G       = [H, C2Q, H*C2Q, H*Q2C]        # [T, 4D]

v2 design (per core, 8 batches), all-bf16 matmuls + bf16 output:
  * Identity  sum_d H[t,d]*w_h[d] = sum_d H[t,d]*(w_h[d]*1)  folds w_h into
    the U-side weights:  S[t,j] = sum_d (U[j,d]*w_hu[d] + w_h[d])*H[t,d]
    + su[j].  No separate sh row, no esh: wq[t] = max_j exp(S) directly.
  * Host supplies H in BOTH layouts as bf16 (t-major for Q2C rhs /
    elementwise G blocks, d-major for the similarity rhs), so the kernel does
    zero H transposes and no H SBUF copies.  U likewise (j-major + d-major).
  * UwT (the similarity lhsT, [d,j]) is built from Ut with one ACT op per
    128-half: scale=w_hu (per-partition), bias=w_h (per-partition).
  * ST[j,t] = UwT.T @ Hdt;  P = exp(ST + su[j]) (ACT bias).  C2Q chunk
    matmuls use P chunks as lhsT directly.  The U ones-column yields the
    softmax normalizer l[t]; the H ones-column yields Wsum for Q2C.
  * max_j P: PE re-transposes P (bf16, 1 cyc/row) and one DVE reduce_max.
  * Q2C: lhsT = wq column broadcast to M=128 (stride-0), so the accumulated
    PSUM [128,258] is the Q2C row already broadcast to every partition;
    normalize+cast in the mandatory PSUM->SBUF ACT copy.
  * G is written to DRAM in bf16, [b, p, g, c, d] layout (4KB contiguous per
    partition per block); the host expands to f32 and un-permutes.  bf16
    rounding is ~4e-3 max rel err vs the 2e-2 gate.
  * Tile emits multi-wait instructions; TRN2 allows 1 wait/instruction, so
    the bacc rust passes (move_matmul_waits_to_ldweights +
    generate_event_semaphores) are run on the traced module before compile.
"""

import os
import sys

sys.path.insert(0, "/opt/trn_rl_repo")

import numpy as np
import ml_dtypes

import concourse.bass as bass
import concourse.mybir as mybir
from concourse import tile

B, T, J, D = 64, 1024, 128, 256
NCORES = 8
BPC = B // NCORES  # batches per core
P = 128
NT = T // P  # 8 t-chunks per batch
DA = 260  # augmented feature dim: [x | 1 | pad(1.0)*3]
F32 = mybir.dt.float32
BF = mybir.dt.bfloat16
AF = mybir.ActivationFunctionType
ALU = mybir.AluOpType
AX = mybir.AxisListType

PHASE = int(os.environ.get("KPHASE", "10"))


def build_kernel(nc, bpc):
    Htd = nc.declare_dram_parameter("Htd", [bpc, P, NT, DA], BF, isOutput=False)
    Hdt = nc.declare_dram_parameter("Hdt", [bpc, P, 2, T], BF, isOutput=False)
    Ubp = nc.declare_dram_parameter("Ub", [bpc, P, DA], BF, isOutput=False)
    Utp = nc.declare_dram_parameter("Ut", [bpc, P, 2, P], BF, isOutput=False)
    wcol_in = nc.declare_dram_parameter("wcol", [P, 2, 2], F32, isOutput=False)
    wub_in = nc.declare_dram_parameter("wub", [P, D], BF, isOutput=False)
    ident_in = nc.declare_dram_parameter("identb", [P, P], BF, isOutput=False)
    # device writes blocks 1..3 only (block 0 = H verbatim, host-assembled)
    G = nc.declare_dram_parameter("G", [bpc, P, 3, NT, D], BF, isOutput=True)

    with tile.TileContext(nc) as tc:
        with (
            tc.tile_pool(name="const", bufs=1) as const_pool,
            tc.tile_pool(name="h", bufs=4) as h_pool,
            tc.tile_pool(name="ht", bufs=4) as ht_pool,
            tc.tile_pool(name="p", bufs=4) as p_pool,
            tc.tile_pool(name="g", bufs=4) as g_pool,
            tc.tile_pool(name="u", bufs=4) as u_pool,
            tc.tile_pool(name="sm", bufs=4) as sm_pool,
            tc.tile_pool(name="stps", bufs=1, space="PSUM") as st_ps,
            tc.tile_pool(name="ptps", bufs=2, space="PSUM") as pt_ps,
            tc.tile_pool(name="cqps", bufs=2, space="PSUM") as cq_ps,
            tc.tile_pool(name="qbps", bufs=2, space="PSUM") as qb_ps,
        ):
            # ---- constants ----
            ident = const_pool.tile([P, P], BF)
            nc.sync.dma_start(ident[:], ident_in[:])
            wcol = const_pool.tile([P, 2, 2], F32)
            nc.sync.dma_start(wcol[:], wcol_in[:])
            wub = const_pool.tile([P, D], BF)
            nc.sync.dma_start(wub[:], wub_in[:])

            # PE p-state heater: ~24 back-to-back dummy transposes during the
            # (otherwise PE-idle) initial input-load window.  >3us of
            # continuous Tensor-engine work ramps the clock toward 2.4 GHz
            # before batch 0's similarity matmuls arrive.
            for i in range(24):
                heat = pt_ps.tile([P, P], BF, tag="pt")
                nc.tensor.transpose(heat[:], ident[:], ident[:])

            for b in range(bpc):
                # ---- load inputs (all loads on the SP queue, no waits) ----
                Ub = u_pool.tile([P, DA], BF)
                nc.sync.dma_start(Ub[:], Ubp[b])
                Ut = u_pool.tile([P, 2, P], BF)
                nc.sync.dma_start(Ut[:], Utp[b])
                HT = ht_pool.tile([P, 2, T], BF)
                nc.sync.dma_start(HT[:], Hdt[b])
                Hn = h_pool.tile([P, NT, DA], BF)
                nc.sync.dma_start(Hn[:], Htd[b])
                # Output writes issue from the (otherwise idle) GpSimd queue
                # so their semaphore waits can't head-of-line-block input
                # loads on SP.
                Gb = G[b]

                if PHASE < 2:
                    continue
                # ---- U-side prep ----
                # UwT[d, j] = Ut[d, j]*w_hu[d] + w_h[d]  (per-partition d)
                UwT = u_pool.tile([P, 2, P], BF)
                for kc in range(2):
                    nc.scalar.activation(
                        UwT[:, kc, :],
                        Ut[:, kc, :],
                        AF.Identity,
                        scale=wcol[:, kc, 0:1],
                        bias=wcol[:, kc, 1:2],
                    )
                # su[j] = U[j]·w_u (on the idle GpSimd engine)
                scr = sm_pool.tile([P, D], BF)
                nc.gpsimd.tensor_mul(scr[:], Ub[:, 0:D], wub[:])
                su = sm_pool.tile([P, 1], F32)
                nc.vector.reduce_sum(su[:], scr[:], axis=AX.X)

                if PHASE < 3:
                    continue
                # ---- similarity: st[j, t] = sum_d UwT[d,j]*HT[d,t] ----
                st = st_ps.tile([P, T], F32, tag="st")
                for kc in range(2):
                    for th in range(2):
                        nc.tensor.matmul(
                            st[:, th * 512 : (th + 1) * 512],
                            UwT[:, kc, :],
                            HT[:, kc, th * 512 : (th + 1) * 512],
                            start=(kc == 0),
                            stop=(kc == 1),
                        )

                if PHASE < 4:
                    continue
                # ---- P = exp(st + su[j]) ----
                Pt = p_pool.tile([P, T], BF)
                nc.scalar.activation(Pt[:], st[:], AF.Exp, bias=su[:], scale=1.0)

                if PHASE < 5:
                    continue
                # ---- wq[t] = max_j P via PE transpose + one DVE reduce ----
                ptp = pt_ps.tile([P, T], BF, tag="pt")
                for c in range(NT):
                    nc.tensor.transpose(
                        ptp[:, c * P : (c + 1) * P],
                        Pt[:, c * P : (c + 1) * P],
                        ident[:],
                    )
                wq = sm_pool.tile([P, NT], BF)
                nc.vector.reduce_max(
                    wq[:].unsqueeze(2),
                    ptp[:].rearrange("p (c j) -> p c j", j=P),
                    axis=AX.X,
                )

                if PHASE < 6:
                    continue
                # ---- Q2C first (ready right after wq): qb[p, d] =
                # sum_t wq[t]*H[t, d], pre-broadcast to all partitions ----
                qb = qb_ps.tile([P, 258], F32, tag="qb")
                for c in range(NT):
                    nc.tensor.matmul(
                        qb[:],
                        wq[:, c : c + 1].broadcast_to((P, P)),
                        Hn[:, c, 0:258],
                        start=(c == 0),
                        stop=(c == NT - 1),
                    )
                rin = sm_pool.tile([P, 1], F32)
                nc.vector.reciprocal(rin[:], qb[:, 256:257])
                q2cb = sm_pool.tile([P, D], BF)
                nc.scalar.activation(q2cb[:], qb[:, 0:D], AF.Copy, scale=rin[:])
                # ---- G3 = H * Q2C (free-dim broadcast of q2cb) ----
                G4 = g_pool.tile([P, NT, D], BF)
                nc.vector.tensor_mul(
                    G4[:],
                    Hn[:, :, 0:D],
                    q2cb[:].unsqueeze(1).broadcast_to((P, NT, D)),
                )
                nc.gpsimd.dma_start(Gb[:, 2, :, :], G4[:])

                # ---- C2Q = softmax_j(S) @ U, per t-chunk ----
                # G12[:, 0] = C2Q (block 1), G12[:, 1] = H*C2Q (block 2):
                # adjacent in the output layout -> one 8KB-per-partition DMA.
                G12 = g_pool.tile([P, 2, NT, D], BF)
                C2Q = G12[:, 0]
                linv = sm_pool.tile([P, NT], F32)
                for c in range(NT):
                    cq = cq_ps.tile([P, 258], F32, tag="cq")
                    nc.tensor.matmul(
                        cq[:],
                        Pt[:, c * P : (c + 1) * P],
                        Ub[:, 0:258],
                        start=True,
                        stop=True,
                    )
                    nc.vector.reciprocal(linv[:, c : c + 1], cq[:, 256:257])
                    if c % 2 == 0 or c == 7:
                        nc.scalar.activation(
                            C2Q[:, c, :],
                            cq[:, 0:D],
                            AF.Copy,
                            scale=linv[:, c : c + 1],
                        )
                    else:
                        nc.vector.tensor_scalar_mul(
                            C2Q[:, c, :], cq[:, 0:D], linv[:, c : c + 1]
                        )

                if PHASE < 7:
                    continue
                nc.gpsimd.dma_start(Gb[:, 0, :, :], C2Q[:])
                # ---- G2 = H * C2Q ----
                nc.vector.tensor_mul(G12[:, 1], Hn[:, :, 0:D], C2Q[:])
                nc.gpsimd.dma_start(Gb[:, 1, :, :], G12[:, 1])


    return nc


_NC_CACHE = {}


def get_nc(bpc=BPC):
    key = (bpc, PHASE)
    if key not in _NC_CACHE:
        import bass_rust as _bass_rust

        nc = bass.Bass()
        build_kernel(nc, bpc)
        # TRN2 allows at most 1 sync wait per instruction (2 on event
        # semaphores); Tile emits more.  These are the bacc lowering passes
        # that legalize the wait lists.
        _bass_rust.move_matmul_waits_to_ldweights(nc.m)
        _bass_rust.generate_event_semaphores(nc)
        # lower bass_isa subclasses (e.g. EVENT_SEMAPHORE_RANGE_CLEAR) into
        # raw InstISA encodings walrus can emit
        mybir.codegen_inst_isa_subclasses(nc)
        _NC_CACHE[key] = nc
    return _NC_CACHE[key]


def _prep_core(Hc, Uc, w_h, w_hu):
    """Host-side layout prep for one core's batches (all bf16)."""
    bpc = Hc.shape[0]
    # Htd[b, p, c, d]: H[b, c*128+p, d], col 256 = 1.0, pad 1.0
    Htd = np.ones((bpc, NT, P, DA), dtype=ml_dtypes.bfloat16)
    Htd[:, :, :, :D] = Hc.reshape(bpc, NT, P, D).astype(ml_dtypes.bfloat16)
    Htd = np.ascontiguousarray(Htd.transpose(0, 2, 1, 3))
    # Hdt[b, pd, kc, t] = H[b, t, kc*128+pd]
    Hdt = np.ascontiguousarray(
        Hc.astype(ml_dtypes.bfloat16)
        .transpose(0, 2, 1)
        .reshape(bpc, 2, P, T)
        .transpose(0, 2, 1, 3)
    )
    # Ub[b, j, d] with ones column
    Ub = np.ones((bpc, P, DA), dtype=ml_dtypes.bfloat16)
    Ub[:, :, :D] = Uc.astype(ml_dtypes.bfloat16)
    # Ut[b, pd, kc, j] = U[b, j, kc*128+pd]
    Ut = np.ascontiguousarray(
        Uc.astype(ml_dtypes.bfloat16)
        .transpose(0, 2, 1)
        .reshape(bpc, 2, P, P)
        .transpose(0, 2, 1, 3)
    )
    return Htd, Hdt, Ub, Ut


def run(inputs, trace=False, **kwargs):
    from concourse.bass_utils import run_bass_kernel_spmd

    nc = get_nc(BPC)
    H = np.asarray(inputs["H"], dtype=np.float32)
    U = np.asarray(inputs["U"], dtype=np.float32)
    w_h = np.asarray(inputs["w_h"], dtype=np.float32)
    w_u = np.asarray(inputs["w_u"], dtype=np.float32)
    w_hu = np.asarray(inputs["w_hu"], dtype=np.float32)
    # wcol[p, kc, 0] = w_hu[kc*128+p] (ACT scale), wcol[p, kc, 1] = w_h (bias)
    wcol = np.stack(
        [w_hu.reshape(2, P).T, w_h.reshape(2, P).T], axis=2
    ).astype(np.float32)
    wcol = np.ascontiguousarray(wcol)
    wub = np.broadcast_to(w_u.astype(ml_dtypes.bfloat16), (P, D)).copy()
    identb = np.eye(P, dtype=ml_dtypes.bfloat16)

    in_maps = []
    for c in range(NCORES):
        Hc = H[c * BPC : (c + 1) * BPC]
        Uc = U[c * BPC : (c + 1) * BPC]
        Htd, Hdt, Ub, Ut = _prep_core(Hc, Uc, w_h, w_hu)
        in_maps.append(
            {
                "Htd": Htd,
                "Hdt": Hdt,
                "Ub": Ub,
                "Ut": Ut,
                "wcol": wcol,
                "wub": wub,
                "identb": identb,
            }
        )
    res = run_bass_kernel_spmd(
        nc, in_maps, core_ids=list(range(NCORES)), trace=trace, **kwargs
    )
    # G_dev[b, p, g, c, d] -> out[b, c*128+p, (g+1)*256+d]; block 0 = H
    out = np.empty((B, T, 4 * D), dtype=np.float32)
    out[:, :, 0:D] = H
    for c in range(NCORES):
        g = np.asarray(res.results[c]["G"]).astype(np.float32)
        out[c * BPC : (c + 1) * BPC, :, D:] = g.transpose(0, 3, 1, 2, 4).reshape(
            BPC, T, 3 * D
        )
    return out, res


def kernel(**inputs):
    out, _ = run(inputs, trace=False)
    return out
